# revision 15
# baseline (speedup 1.0000x reference)
"""Per-pixel kernel-lookup conv for trn2, data-parallel over batch on 8 cores.

Per core (one image): host-side im2col (144 tap-rows) -> conv against all
128 kernels via 2 accumulating fp16 matmuls (K=128 + K=16), multiply by a
host-precomputed fp8 one-hot mask (DVE/Pool alternating), ones-matmul
partition-reduce (lagged 2 chunks to keep the PE stream dense), direct
PSUM->HBM DMA evacuation.
"""
import numpy as np

RAST = 126 * 128  # output raster, 126 rows padded to 128 wide
NCH = (RAST + 511) // 512  # 32 chunks of <=512 cols
_NC_CACHE = {}


def _split_waits_json(bj: bytes) -> bytes:
    """Walrus rejects >4 sync-waits per instruction (and ~2 on Matmult).
    Split excess waits onto same-engine NoOps inserted just before."""
    import json

    j = json.loads(bj)
    ctr = 0
    for f in j["functions"]:
        for bb in f["blocks"]:
            out = []
            for inst in bb["instructions"]:
                si = inst.get("sync_info")
                cap = 1
                waits = (si or {}).get("on_wait") or []
                if len(waits) > cap:
                    extra, keep = waits[:-cap], waits[-cap:]
                    for g in range(0, len(extra), 1):
                        ctr += 1
                        out.append({
                            "debug": inst.get("debug", 0),
                            "engine": inst["engine"],
                            "ins": [],
                            "name": f"WS-{ctr}",
                            "opcode": "NoOp",
                            "outs": [],
                            "sync_info": {"on_update": [],
                                          "on_wait": extra[g:g + 1]},
                        })
                    si["on_wait"] = keep
                out.append(inst)
            bb["instructions"] = out
    return json.dumps(j).encode()


def _build_nc():
    from contextlib import ExitStack

    import concourse.bass as bass
    import concourse.tile as tile
    from concourse import mybir

    F32 = mybir.dt.float32
    F16 = mybir.dt.float16
    F8 = mybir.dt.float8e4

    nc = bass.Bass(trn_type="TRN2", target_bir_lowering=False)
    bufA = nc.dram_tensor("bufA", [128, RAST], F16, kind="ExternalInput")
    bufB = nc.dram_tensor("bufB", [16, RAST], F16, kind="ExternalInput")
    oh = nc.dram_tensor("oh", [128, RAST], F8, kind="ExternalInput")
    w8 = nc.dram_tensor("w8", [128, 128], F16, kind="ExternalInput")
    w1 = nc.dram_tensor("w1", [16, 128], F16, kind="ExternalInput")
    o = nc.dram_tensor("o", [1, 16384], F32, kind="ExternalOutput")

    # SBUF input sub-tiles in 512-col multiples, graduated small-first so
    # chunk 0's inputs land ASAP; each tile is written by exactly one DMA.
    # Tails beyond RAST are zero-filled so every chunk runs a full 512 cols.
    A_WS = [512] * 8 + [1024] * 12
    O_WS = [512] * 4 + [2048] * 7
    B_WS = [2048, 2048, 12288]

    with tile.TileContext(nc) as tc, ExitStack() as ctx:
        sb = ctx.enter_context(tc.tile_pool(name="sb", bufs=1))
        msk = ctx.enter_context(tc.tile_pool(name="msk", bufs=4))
        psc_pool = ctx.enter_context(tc.tile_pool(name="psc", bufs=4, space="PSUM"))
        pso_pool = ctx.enter_context(tc.tile_pool(name="pso", bufs=2, space="PSUM"))

        ones = sb.tile([128, 1], F16)
        nc.vector.memset(ones[:], 1.0)
        w8_t = sb.tile([128, 128], F16)
        nc.gpsimd.dma_start(w8_t[:], w8.ap())
        w1_t = sb.tile([16, 128], F16)
        nc.gpsimd.dma_start(w1_t[:], w1.ap())

        def load_tiles(widths, dram, parts, dt, engs, tag):
            tiles, c0 = [], 0
            for i, w in enumerate(widths):
                c1 = min(c0 + w, RAST)
                t = sb.tile([parts, w], dt, name=f"{tag}{i}")
                if c1 - c0 < w:
                    nc.vector.memset(t[:, c1 - c0:], 0.0)
                engs[i % len(engs)].dma_start(t[:, :c1 - c0],
                                              dram.ap()[:, c0:c1])
                tiles.append((c0, w, t))
                c0 += w
            return tiles

        def pick(tiles, n0):
            for c0, w, t in tiles:
                if c0 <= n0 < c0 + w:
                    return t, n0 - c0
            raise AssertionError(n0)

        bufB_t = load_tiles(B_WS, bufB, 16, F16, [nc.scalar], "bufB")
        bufA_t = load_tiles(A_WS, bufA, 128, F16, [nc.sync, nc.scalar], "bufA")
        oh_t = load_tiles(O_WS, oh, 128, F8, [nc.gpsimd], "oh")

        out_sb = sb.tile([1, 16384], F32)

        psc_l = [None] * NCH
        m_l = [None] * NCH
        for it in range(NCH + 2):
            if it < NCH:
                c = it
                n0 = c * 512
                ta, ao = pick(bufA_t, n0)
                tb, bo = pick(bufB_t, n0)
                to, oo = pick(oh_t, n0)
                psc = psc_pool.tile([128, 512], F32)
                psc_l[c] = psc
                nc.tensor.matmul(psc[:], lhsT=w8_t[:],
                                 rhs=ta[:, ao:ao + 512],
                                 start=True, stop=False)
                nc.tensor.matmul(psc[:], lhsT=w1_t[:],
                                 rhs=tb[:, bo:bo + 512],
                                 start=False, stop=True)
                m = msk.tile([128, 512], F16)
                m_l[c] = m
                nc.vector.tensor_tensor(
                    out=m[:], in0=to[:, oo:oo + 512],
                    in1=psc[:], op=mybir.AluOpType.mult)
            r = it - 2
            if r >= 0:
                if r % 2 == 0:
                    pso = pso_pool.tile([1, 1024], F32)
                off = (r % 2) * 512
                nc.tensor.matmul(pso[:, off:off + 512], lhsT=ones[:],
                                 rhs=m_l[r][:], start=True, stop=True)
                if r % 2 == 1:
                    g0 = (r - 1) * 512
                    nc.scalar.copy(out_sb[0:1, g0:g0 + 1024],
                                   pso[0:1, 0:1024])
                    if r % 4 == 3:
                        nc.scalar.dma_start(o.ap()[:, g0 - 1024:g0 + 1024],
                                            out_sb[0:1, g0 - 1024:g0 + 1024])

    orig = nc.to_json_bytes
    nc.to_json_bytes = lambda: _split_waits_json(orig())
    return nc


def _get_nc():
    if "nc" not in _NC_CACHE:
        _NC_CACHE["nc"] = _build_nc()
    return _NC_CACHE["nc"]


def _in_maps(data, kernel_idx, weights):
    import ml_dtypes

    B = data.shape[0]
    # w8[(dy*3+dx)*16+c, j] = weights[j, c, dy, dx] for taps 0..7; w1 tap 8
    wt = np.ascontiguousarray(
        np.transpose(weights, (2, 3, 1, 0)).reshape(144, 128)
    ).astype(np.float16)
    w8 = np.ascontiguousarray(wt[:128])
    w1 = np.ascontiguousarray(wt[128:])
    jj = np.arange(128, dtype=np.int32).reshape(128, 1)
    maps = []
    for b in range(B):
        flat = np.zeros((16, 128 * 128 + 384), dtype=np.float16)
        flat[:, :128 * 128] = data[b].astype(np.float16).reshape(16, -1)
        # imcol[(dy*3+dx)*16+c, h*128+w] = data[c, h+dy, w+dx]
        imcol = np.empty((144, RAST), dtype=np.float16)
        for t in range(9):
            dy, dx = divmod(t, 3)
            off = dy * 128 + dx
            imcol[t * 16:(t + 1) * 16] = flat[:, off:off + RAST]
        idxr = np.full((126, 128), 255, dtype=np.int32)
        idxr[:, :126] = kernel_idx[b].astype(np.int32)
        ohb = (idxr.reshape(1, RAST) == jj).astype(ml_dtypes.float8_e4m3)
        maps.append({
            "bufA": np.ascontiguousarray(imcol[:128]),
            "bufB": np.ascontiguousarray(imcol[128:]),
            "oh": ohb,
            "w8": w8,
            "w1": w1,
        })
    return maps


def kernel(data, kernel_idx, weights, _trace=False):
    from concourse.bass_utils import run_bass_kernel_spmd

    data = np.asarray(data, dtype=np.float32)
    kernel_idx = np.asarray(kernel_idx)
    weights = np.asarray(weights, dtype=np.float32)
    B = data.shape[0]
    nc = _get_nc()
    res = run_bass_kernel_spmd(nc, _in_maps(data, kernel_idx, weights),
                               core_ids=list(range(B)), trace=_trace)
    out = np.stack([
        r["o"].reshape(16384)[:RAST].reshape(126, 128)[:, :126]
        for r in res.results
    ])
    if _trace:
        return out.astype(np.float32), res
    return out.astype(np.float32)


# revision 16
# speedup vs baseline: 1.1175x; 1.1175x over previous
"""Per-pixel kernel-lookup conv for trn2, data-parallel over batch on 8 cores.

Per core (one image): host-side im2col (144 tap-rows) -> conv against all
128 kernels via 2 accumulating fp16 matmuls (K=128 + K=16), multiply by a
host-precomputed fp8 one-hot mask (DVE/Pool alternating), ones-matmul
partition-reduce (lagged 2 chunks to keep the PE stream dense), direct
PSUM->HBM DMA evacuation.
"""
import numpy as np

RAST = 126 * 128  # output raster, 126 rows padded to 128 wide
NCH = (RAST + 511) // 512  # 32 chunks of <=512 cols
_NC_CACHE = {}


def _split_waits_json(bj: bytes) -> bytes:
    """Walrus rejects >4 sync-waits per instruction (and ~2 on Matmult).
    Split excess waits onto same-engine NoOps inserted just before."""
    import json

    j = json.loads(bj)
    ctr = 0
    for f in j["functions"]:
        for bb in f["blocks"]:
            out = []
            for inst in bb["instructions"]:
                si = inst.get("sync_info")
                cap = 1
                waits = (si or {}).get("on_wait") or []
                if len(waits) > cap:
                    extra, keep = waits[:-cap], waits[-cap:]
                    for g in range(0, len(extra), 1):
                        ctr += 1
                        out.append({
                            "debug": inst.get("debug", 0),
                            "engine": inst["engine"],
                            "ins": [],
                            "name": f"WS-{ctr}",
                            "opcode": "NoOp",
                            "outs": [],
                            "sync_info": {"on_update": [],
                                          "on_wait": extra[g:g + 1]},
                        })
                    si["on_wait"] = keep
                out.append(inst)
            bb["instructions"] = out
    return json.dumps(j).encode()


def _build_nc():
    from contextlib import ExitStack

    import concourse.bass as bass
    import concourse.tile as tile
    from concourse import mybir

    F32 = mybir.dt.float32
    F16 = mybir.dt.float16
    F8 = mybir.dt.float8e4

    nc = bass.Bass(trn_type="TRN2", target_bir_lowering=False)
    bufA = nc.dram_tensor("bufA", [128, RAST], F16, kind="ExternalInput")
    bufB = nc.dram_tensor("bufB", [16, RAST], F16, kind="ExternalInput")
    oh = nc.dram_tensor("oh", [128, RAST], F8, kind="ExternalInput")
    w8 = nc.dram_tensor("w8", [128, 128], F16, kind="ExternalInput")
    w1 = nc.dram_tensor("w1", [16, 128], F16, kind="ExternalInput")
    o = nc.dram_tensor("o", [1, 16384], F32, kind="ExternalOutput")

    # SBUF input sub-tiles in 512-col multiples, graduated small-first so
    # chunk 0's inputs land ASAP; each tile is written by exactly one DMA.
    # Tails beyond RAST are zero-filled so every chunk runs a full 512 cols.
    A_WS = [1024] * 16
    O_WS = [2048] * 8
    B_WS = [8192] * 2

    with tile.TileContext(nc) as tc, ExitStack() as ctx:
        sb = ctx.enter_context(tc.tile_pool(name="sb", bufs=1))
        msk = ctx.enter_context(tc.tile_pool(name="msk", bufs=4))
        psc_pool = ctx.enter_context(tc.tile_pool(name="psc", bufs=4, space="PSUM"))
        pso_pool = ctx.enter_context(tc.tile_pool(name="pso", bufs=2, space="PSUM"))

        ones = sb.tile([128, 1], F16)
        nc.vector.memset(ones[:], 1.0)
        w8_t = sb.tile([128, 128], F16)
        nc.gpsimd.dma_start(w8_t[:], w8.ap())
        w1_t = sb.tile([16, 128], F16)
        nc.gpsimd.dma_start(w1_t[:], w1.ap())

        def load_tiles(widths, dram, parts, dt, engs, tag):
            tiles, c0 = [], 0
            for i, w in enumerate(widths):
                c1 = min(c0 + w, RAST)
                t = sb.tile([parts, w], dt, name=f"{tag}{i}")
                if c1 - c0 < w:
                    nc.vector.memset(t[:, c1 - c0:], 0.0)
                engs[i % len(engs)].dma_start(t[:, :c1 - c0],
                                              dram.ap()[:, c0:c1])
                tiles.append((c0, w, t))
                c0 += w
            return tiles

        def pick(tiles, n0):
            for c0, w, t in tiles:
                if c0 <= n0 < c0 + w:
                    return t, n0 - c0
            raise AssertionError(n0)

        bufA_t = load_tiles(A_WS, bufA, 128, F16, [nc.sync], "bufA")
        oh_t = load_tiles(O_WS, oh, 128, F8, [nc.scalar], "oh")
        bufB_t = load_tiles(B_WS, bufB, 16, F16, [nc.gpsimd], "bufB")

        out_sb = sb.tile([1, 16384], F32)

        psc_l = [None] * NCH
        m_l = [None] * NCH
        for it in range(NCH + 2):
            if it < NCH:
                c = it
                n0 = c * 512
                ta, ao = pick(bufA_t, n0)
                tb, bo = pick(bufB_t, n0)
                to, oo = pick(oh_t, n0)
                psc = psc_pool.tile([128, 512], F32)
                psc_l[c] = psc
                nc.tensor.matmul(psc[:], lhsT=w8_t[:],
                                 rhs=ta[:, ao:ao + 512],
                                 start=True, stop=False)
                nc.tensor.matmul(psc[:], lhsT=w1_t[:],
                                 rhs=tb[:, bo:bo + 512],
                                 start=False, stop=True)
                m = msk.tile([128, 512], F16)
                m_l[c] = m
                nc.vector.tensor_tensor(
                    out=m[:], in0=to[:, oo:oo + 512],
                    in1=psc[:], op=mybir.AluOpType.mult)
            r = it - 2
            if r >= 0:
                if r % 2 == 0:
                    pso = pso_pool.tile([1, 1024], F32)
                off = (r % 2) * 512
                nc.tensor.matmul(pso[:, off:off + 512], lhsT=ones[:],
                                 rhs=m_l[r][:], start=True, stop=True)
                if r % 2 == 1:
                    g0 = (r - 1) * 512
                    nc.scalar.copy(out_sb[0:1, g0:g0 + 1024],
                                   pso[0:1, 0:1024])
                    if r % 8 == 7:
                        nc.scalar.dma_start(o.ap()[:, g0 - 3072:g0 + 1024],
                                            out_sb[0:1, g0 - 3072:g0 + 1024])

    orig = nc.to_json_bytes
    nc.to_json_bytes = lambda: _split_waits_json(orig())
    return nc


def _get_nc():
    if "nc" not in _NC_CACHE:
        _NC_CACHE["nc"] = _build_nc()
    return _NC_CACHE["nc"]


def _in_maps(data, kernel_idx, weights):
    import ml_dtypes

    B = data.shape[0]
    # w8[(dy*3+dx)*16+c, j] = weights[j, c, dy, dx] for taps 0..7; w1 tap 8
    wt = np.ascontiguousarray(
        np.transpose(weights, (2, 3, 1, 0)).reshape(144, 128)
    ).astype(np.float16)
    w8 = np.ascontiguousarray(wt[:128])
    w1 = np.ascontiguousarray(wt[128:])
    jj = np.arange(128, dtype=np.int32).reshape(128, 1)
    maps = []
    for b in range(B):
        flat = np.zeros((16, 128 * 128 + 384), dtype=np.float16)
        flat[:, :128 * 128] = data[b].astype(np.float16).reshape(16, -1)
        # imcol[(dy*3+dx)*16+c, h*128+w] = data[c, h+dy, w+dx]
        imcol = np.empty((144, RAST), dtype=np.float16)
        for t in range(9):
            dy, dx = divmod(t, 3)
            off = dy * 128 + dx
            imcol[t * 16:(t + 1) * 16] = flat[:, off:off + RAST]
        idxr = np.full((126, 128), 255, dtype=np.int32)
        idxr[:, :126] = kernel_idx[b].astype(np.int32)
        ohb = (idxr.reshape(1, RAST) == jj).astype(ml_dtypes.float8_e4m3)
        maps.append({
            "bufA": np.ascontiguousarray(imcol[:128]),
            "bufB": np.ascontiguousarray(imcol[128:]),
            "oh": ohb,
            "w8": w8,
            "w1": w1,
        })
    return maps


def kernel(data, kernel_idx, weights, _trace=False):
    from concourse.bass_utils import run_bass_kernel_spmd

    data = np.asarray(data, dtype=np.float32)
    kernel_idx = np.asarray(kernel_idx)
    weights = np.asarray(weights, dtype=np.float32)
    B = data.shape[0]
    nc = _get_nc()
    res = run_bass_kernel_spmd(nc, _in_maps(data, kernel_idx, weights),
                               core_ids=list(range(B)), trace=_trace)
    out = np.stack([
        r["o"].reshape(16384)[:RAST].reshape(126, 128)[:, :126]
        for r in res.results
    ])
    if _trace:
        return out.astype(np.float32), res
    return out.astype(np.float32)
